# revision 13
# baseline (speedup 1.0000x reference)
"""Trainium2 Bass kernel for BipolarSAE (top-k masking sparse autoencoder).

reference:
    pre = x @ W_enc.T + b_enc          # [N, 4096]
    keep top-32 of |pre| per row (mask), f = pre * mask
    recon = f @ W_dec.T + b_dec        # [N, 768]
    returns (recon, f)

Strategy (8 NeuronCores, data-parallel over the 32768 tokens, 4096 each):
  Phase 1 (W_enc resident): encoder matmul via fp16 hi/lo decomposition —
    x = x_hi + x_lo, W = W_hi + W_lo (fp16 splits, products exact in fp32
    PSUM accumulation), pre = x_hi@W_hi + x_hi@W_lo + x_lo@W_hi. Error
    ~2^-22 relative, matching native fp32, at 3 bf16-rate passes instead
    of fp32's 4 cycles/row. Selection needs this precision: the top-32
    boundary gap can be ~1e-7.
    ACT evacuates PSUM as pre (fp32) and pre^2; VectorE extracts the
    32nd-largest square via 4x max8 + 3x match_replace, then one
    scalar_tensor_tensor applies the threshold mask: f = (sq>=tau^2)*pre.
    f goes to DRAM in fp32 (output) and fp16 (scratch for phase 2).
  Phase 2 (W_dec resident, fp16): per block one DMA-transpose loads
    f^T (features-on-partitions) from the fp16 scratch; decoder matmul
    all-fp16 (decoder precision is not selection-critical).
Biases are folded in as K=1 matmuls with a ones row-vector (b_enc in
fp16 hi+lo for exactness; b_dec likewise).
"""

import os
import sys

sys.path.insert(0, "/opt/trn_rl_repo")

import numpy as np

import concourse.bacc as bacc
import concourse.bass as bass
import concourse.mybir as mybir
import concourse.tile as tile
from concourse.bass import ts
from concourse.tile_rust import add_dep_helper

P = 128
D_IN = 768
D_OUT = 4096
K_TOP = 32
N_TOKENS = 32768
N_CORES = 8

KO = D_IN // P  # 6 contraction chunks (encoder)
NSL = 8  # encoder feature slices
SL = D_OUT // NSL  # 512
FC = D_OUT // P  # 32 feature chunks (decoder contraction)

LAST_RESULTS = None  # test harness reads exec_time_ns from here

f32 = mybir.dt.float32
f16 = mybir.dt.float16


def build(t_core: int) -> bacc.Bacc:
    nblk = t_core // P
    nc = bacc.Bacc("TRN2", target_bir_lowering=False, debug=False)

    xThl = nc.declare_dram_parameter("xThl", [2 * D_IN, t_core], f16, isOutput=False)
    wencTh = nc.declare_dram_parameter("wencTh", [D_IN, D_OUT], f16, isOutput=False)
    wencTl = nc.declare_dram_parameter("wencTl", [D_IN, D_OUT], f16, isOutput=False)
    bench = nc.declare_dram_parameter("bench", [1, D_OUT], f16, isOutput=False)
    bencl = nc.declare_dram_parameter("bencl", [1, D_OUT], f16, isOutput=False)
    wdecT = nc.declare_dram_parameter("wdecT", [D_OUT, D_IN], f16, isOutput=False)
    bdech = nc.declare_dram_parameter("bdech", [1, D_IN], f16, isOutput=False)
    bdecl = nc.declare_dram_parameter("bdecl", [1, D_IN], f16, isOutput=False)

    f_out = nc.declare_dram_parameter("f", [t_core, D_OUT], f32, isOutput=True)
    f16_scr = nc.dram_tensor("f16scr", [t_core, D_OUT], f16)
    recon_out = nc.declare_dram_parameter("recon", [t_core, D_IN], f32, isOutput=True)

    xThl_t = xThl.ap().rearrange("(o p) t -> p o t", p=P)  # [128, 12, t]

    with tile.TileContext(nc) as tc:
        # ---------------- Phase 1: encode + top-k mask ----------------
        with (
            tc.tile_pool(name="p1w", bufs=1) as wpool,
            tc.tile_pool(name="p1xt", bufs=2) as xtpool,
            tc.tile_pool(name="p1dbl", bufs=2) as dpool,
            tc.tile_pool(name="p1sgl", bufs=1) as spool,
            tc.tile_pool(name="p1ps", bufs=4, space="PSUM") as ppool,
        ):
            bhl_sb = wpool.tile([2, D_OUT], f16)
            nc.sync.dma_start(bhl_sb[0:1, :], bench.ap())
            nc.sync.dma_start(bhl_sb[1:2, :], bencl.ap())
            ones_sb = wpool.tile([2, P], f16)
            nc.vector.memset(ones_sb[:], 1.0)
            wh_s = []
            wl_s = []
            for s in range(NSL):
                wh = wpool.tile([P, KO, SL], f16, name=f"whs{s}")
                nc.sync.dma_start(
                    wh[:], wencTh.ap().rearrange("(o p) f -> p o f", p=P)[:, :, ts(s, SL)]
                )
                wh_s.append(wh)
                wl = wpool.tile([P, KO, SL], f16, name=f"wls{s}")
                nc.sync.dma_start(
                    wl[:], wencTl.ap().rearrange("(o p) f -> p o f", p=P)[:, :, ts(s, SL)]
                )
                wl_s.append(wl)
            f16_dmas = []
            for b in range(nblk):
                xhl = xtpool.tile([P, 2 * KO, P], f16, name=f"xhl{b}", tag="xhl")
                nc.scalar.dma_start(xhl[:], xThl_t[:, :, ts(b, P)])
                xh = xhl[:, :KO, :]
                xl = xhl[:, KO:, :]

                pre = dpool.tile([P, D_OUT], f32, name=f"pre{b}", tag="pre")
                sq = dpool.tile([P, D_OUT], f32, name=f"sq{b}", tag="sq")
                for g in range(NSL // 2):
                    s0, s1 = 2 * g, 2 * g + 1
                    psA = ppool.tile([P, SL], f32, name=f"eps{b}_{s0}", tag="eps")
                    psB = ppool.tile([P, SL], f32, name=f"eps{b}_{s1}", tag="eps")
                    nc.tensor.matmul(
                        psA[:], ones_sb[:], bhl_sb[:, ts(s0, SL)], start=True, stop=False
                    )
                    nc.tensor.matmul(
                        psB[:], ones_sb[:], bhl_sb[:, ts(s1, SL)], start=True, stop=False
                    )
                    # each stationary xt chunk feeds both slices of the pair
                    for pi, (xt_c, whl) in enumerate(
                        ((xh, wh_s), (xh, wl_s), (xl, wh_s))
                    ):
                        for ko in range(KO):
                            last = pi == 2 and ko == KO - 1
                            nc.tensor.matmul(
                                psA[:], xt_c[:, ko, :], whl[s0][:, ko, :],
                                start=False, stop=last,
                            )
                            nc.tensor.matmul(
                                psB[:], xt_c[:, ko, :], whl[s1][:, ko, :],
                                start=False, stop=last,
                            )
                    for s, ps in ((s0, psA), (s1, psB)):
                        nc.scalar.activation(
                            sq[:, ts(s, SL)], ps[:], mybir.ActivationFunctionType.Square
                        )
                        nc.scalar.copy(pre[:, ts(s, SL)], ps[:])

                # top-32 threshold (on squares): 4x max8 + 3x match_replace.
                # Round-1 max8 runs split in halves so the first half starts
                # as soon as encoder slices 0-3 are evacuated.
                zap = spool.tile([P, D_OUT], f32, name=f"zap{b}", tag="zap")
                m8 = dpool.tile([P, 4, 8], f32, name=f"m8{b}", tag="m8")
                m16 = dpool.tile([P, 16], f32, name=f"m16{b}", tag="m16")
                nc.vector.max(out=m16[:, :8], in_=sq[:, : D_OUT // 2])
                nc.vector.max(out=m16[:, 8:], in_=sq[:, D_OUT // 2 :])
                nc.vector.max(out=m8[:, 0, :], in_=m16[:])
                nc.vector.match_replace(
                    out=zap[:], in_to_replace=m8[:, 0, :], in_values=sq[:], imm_value=-1.0
                )
                for r in range(1, 4):
                    nc.vector.max(out=m8[:, r, :], in_=zap[:])
                    if r < 3:
                        nc.vector.match_replace(
                            out=zap[:],
                            in_to_replace=m8[:, r, :],
                            in_values=zap[:],
                            imm_value=-1.0,
                        )

                f_sb = pre
                nc.vector.scalar_tensor_tensor(
                    out=f_sb[:],
                    in0=sq[:],
                    scalar=m8[:, 3, 7:8],
                    in1=pre[:],
                    op0=mybir.AluOpType.is_ge,
                    op1=mybir.AluOpType.mult,
                )
                nc.scalar.dma_start(f_out.ap()[ts(b, P), :], f_sb[:])
                f16t = spool.tile([P, D_OUT], f16, name=f"f16t{b}", tag="f16t")
                nc.scalar.copy(f16t[:], f_sb[:])
                f16_dma = nc.scalar.dma_start(f16_scr.ap()[ts(b, P), :], f16t[:])
                f16_dmas.append(f16_dma)

        # ---------------- Phase 2: decode ----------------
        with (
            tc.tile_pool(name="p2w", bufs=1) as wpool2,
            tc.tile_pool(name="p2ft", bufs=3) as ftpool,
            tc.tile_pool(name="p2rec", bufs=3) as recpool,
            tc.tile_pool(name="p2ps", bufs=4, space="PSUM") as rpspool,
        ):
            wdec_g = []
            for g4 in range(4):
                wg = wpool2.tile([P, FC // 4, D_IN], f16, name=f"wdg{g4}")
                nc.sync.dma_start(
                    wg[:],
                    wdecT.ap().rearrange("(o p) d -> p o d", p=P)[
                        :, ts(g4, FC // 4), :
                    ],
                )
                wdec_g.append(wg)
            bdhl_sb = wpool2.tile([2, D_IN], f16)
            nc.sync.dma_start(bdhl_sb[0:1, :], bdech.ap())
            nc.sync.dma_start(bdhl_sb[1:2, :], bdecl.ap())
            ones2 = wpool2.tile([2, P], f16)
            nc.vector.memset(ones2[:], 1.0)

            for b in range(nblk):
                fT = ftpool.tile([P, FC, P], f16, name=f"fT{b}", tag="fT")
                tr_eng = nc.sync if b % 2 == 0 else nc.scalar
                tr = tr_eng.dma_start_transpose(fT[:], f16_scr.ap()[ts(b, P), :])
                add_dep_helper(
                    tr.ins, f16_dmas[b].ins, sync=True, reason="f16 scratch RAW"
                )

                rps = rpspool.tile([P, D_IN], f32, name=f"rps{b}", tag="rps")
                # chunk-outer: both output ranges stream from one stationary
                nc.tensor.matmul(
                    rps[:, 0:512], ones2[:], bdhl_sb[:, 0:512], start=True, stop=False
                )
                nc.tensor.matmul(
                    rps[:, 512:768], ones2[:], bdhl_sb[:, 512:768], start=True, stop=False
                )
                for c in range(FC):
                    for n0, n1 in ((0, 512), (512, 768)):
                        nc.tensor.matmul(
                            rps[:, n0:n1],
                            fT[:, c, :],
                            wdec_g[c // 8][:, c % 8, n0:n1],
                            start=False,
                            stop=(c == FC - 1),
                        )
                rec = recpool.tile([P, D_IN], f32, name=f"rec{b}", tag="rec")
                nc.scalar.copy(rec[:], rps[:])
                nc.scalar.dma_start(recon_out.ap()[ts(b, P), :], rec[:])

    nc.compile()
    return nc


_BUILT = {}


def _get_built(t_core: int):
    if t_core not in _BUILT:
        _BUILT[t_core] = build(t_core)
    return _BUILT[t_core]


def _split16(a):
    hi = a.astype(np.float16)
    lo = (a - hi.astype(np.float32)).astype(np.float16)
    return hi, lo


def _install_ntff_shim():
    """The image's antenv lacks axon_hooks; synthesize it from trn_agent_boot
    so run_bass_kernel_spmd(trace=True) can capture NTFF profiles."""
    import types

    if "antenv.axon_hooks" in sys.modules:
        return
    try:
        from trn_agent_boot.trn_boot import _ntff_profile_via_ctypes

        hook = _ntff_profile_via_ctypes("/opt/axon/libaxon_pjrt.so")
        mod = types.ModuleType("antenv.axon_hooks")
        mod.get_axon_ntff_profile_hook = lambda: hook
        sys.modules["antenv.axon_hooks"] = mod
    except Exception:
        pass


def _enable_ldw_opt():
    from concourse import bass_utils as bu

    if getattr(bu, "_ldw_opt_patched", False):
        return
    orig = bu.run_command

    def patched(cmd, *a, **kw):
        if isinstance(cmd, list):
            cmd = [
                "--enable-ldw-opt=true" if c == "--enable-ldw-opt=false" else c
                for c in cmd
            ]
        return orig(cmd, *a, **kw)

    bu.run_command = patched
    bu._ldw_opt_patched = True


def kernel(x, W_enc, b_enc, W_dec, b_dec):
    global LAST_RESULTS
    from concourse.bass_utils import run_bass_kernel_spmd

    if os.environ.get("SAE_LDW_OPT"):
        _enable_ldw_opt()

    if os.environ.get("SAE_TRACE"):
        _install_ntff_shim()

    x = np.asarray(x, dtype=np.float32)
    W_enc = np.asarray(W_enc, dtype=np.float32)
    b_enc = np.asarray(b_enc, dtype=np.float32)
    W_dec = np.asarray(W_dec, dtype=np.float32)
    b_dec = np.asarray(b_dec, dtype=np.float32)

    n_tokens = x.shape[0]
    t_core = n_tokens // N_CORES
    nc = _get_built(t_core)

    xT = np.ascontiguousarray(x.T)  # [768, N]
    xTh, xTl = _split16(xT)
    xThl = np.concatenate([xTh, xTl], axis=0)  # [1536, N]
    wencTh, wencTl = _split16(np.ascontiguousarray(W_enc.T))
    bench_, bencl_ = _split16(b_enc[None, :])
    wdecT = np.ascontiguousarray(W_dec.T).astype(np.float16)
    bdech_, bdecl_ = _split16(b_dec[None, :])

    in_maps = [
        {
            "xThl": np.ascontiguousarray(xThl[:, i * t_core : (i + 1) * t_core]),
            "wencTh": wencTh,
            "wencTl": wencTl,
            "bench": bench_,
            "bencl": bencl_,
            "wdecT": wdecT,
            "bdech": bdech_,
            "bdecl": bdecl_,
        }
        for i in range(N_CORES)
    ]

    res = run_bass_kernel_spmd(
        nc,
        in_maps,
        list(range(N_CORES)),
        trace=bool(os.environ.get("SAE_TRACE")),
    )
    LAST_RESULTS = res

    recon = np.concatenate([res.results[i]["recon"] for i in range(N_CORES)], axis=0)
    f = np.concatenate([res.results[i]["f"] for i in range(N_CORES)], axis=0)
    return recon, f


# revision 14
# speedup vs baseline: 1.0401x; 1.0401x over previous
"""Trainium2 Bass kernel for BipolarSAE (top-k masking sparse autoencoder).

reference:
    pre = x @ W_enc.T + b_enc          # [N, 4096]
    keep top-32 of |pre| per row (mask), f = pre * mask
    recon = f @ W_dec.T + b_dec        # [N, 768]
    returns (recon, f)

Strategy (8 NeuronCores, data-parallel over the 32768 tokens, 4096 each):
  Phase 1 (W_enc resident): encoder matmul via fp16 hi/lo decomposition —
    x = x_hi + x_lo, W = W_hi + W_lo (fp16 splits, products exact in fp32
    PSUM accumulation), pre = x_hi@W_hi + x_hi@W_lo + x_lo@W_hi. Error
    ~2^-22 relative, matching native fp32, at 3 bf16-rate passes instead
    of fp32's 4 cycles/row. Selection needs this precision: the top-32
    boundary gap can be ~1e-7.
    ACT evacuates PSUM as pre (fp32) and pre^2; VectorE extracts the
    32nd-largest square via 4x max8 + 3x match_replace, then one
    scalar_tensor_tensor applies the threshold mask: f = (sq>=tau^2)*pre.
    f goes to DRAM in fp32 (output) and fp16 (scratch for phase 2).
  Phase 2 (W_dec resident, fp16): per block one DMA-transpose loads
    f^T (features-on-partitions) from the fp16 scratch; decoder matmul
    all-fp16 (decoder precision is not selection-critical).
Biases are folded in as K=1 matmuls with a ones row-vector (b_enc in
fp16 hi+lo for exactness; b_dec likewise).
"""

import os
import sys

sys.path.insert(0, "/opt/trn_rl_repo")

import numpy as np

import concourse.bacc as bacc
import concourse.bass as bass
import concourse.mybir as mybir
import concourse.tile as tile
from concourse.bass import ts
from concourse.tile_rust import add_dep_helper

P = 128
D_IN = 768
D_OUT = 4096
K_TOP = 32
N_TOKENS = 32768
N_CORES = 8

KO = D_IN // P  # 6 contraction chunks (encoder)
NSL = 8  # encoder feature slices
SL = D_OUT // NSL  # 512
FC = D_OUT // P  # 32 feature chunks (decoder contraction)

LAST_RESULTS = None  # test harness reads exec_time_ns from here

f32 = mybir.dt.float32
f16 = mybir.dt.float16


def build(t_core: int) -> bacc.Bacc:
    nblk = t_core // P
    nc = bacc.Bacc("TRN2", target_bir_lowering=False, debug=False)

    xThl = nc.declare_dram_parameter("xThl", [2 * D_IN, t_core], f16, isOutput=False)
    wencTh = nc.declare_dram_parameter("wencTh", [D_IN, D_OUT], f16, isOutput=False)
    wencTl = nc.declare_dram_parameter("wencTl", [D_IN, D_OUT], f16, isOutput=False)
    bench = nc.declare_dram_parameter("bench", [1, D_OUT], f16, isOutput=False)
    bencl = nc.declare_dram_parameter("bencl", [1, D_OUT], f16, isOutput=False)
    wdecT = nc.declare_dram_parameter("wdecT", [D_OUT, D_IN], f16, isOutput=False)
    bdech = nc.declare_dram_parameter("bdech", [1, D_IN], f16, isOutput=False)
    bdecl = nc.declare_dram_parameter("bdecl", [1, D_IN], f16, isOutput=False)

    f_out = nc.declare_dram_parameter("f", [t_core, D_OUT], f32, isOutput=True)
    f16_scr = nc.dram_tensor("f16scr", [t_core, D_OUT], f16)
    recon_out = nc.declare_dram_parameter("recon", [t_core, D_IN], f32, isOutput=True)

    xThl_t = xThl.ap().rearrange("(o p) t -> p o t", p=P)  # [128, 12, t]

    with tile.TileContext(nc) as tc:
        # ---------------- Phase 1: encode + top-k mask ----------------
        with (
            tc.tile_pool(name="p1w", bufs=1) as wpool,
            tc.tile_pool(name="p1xt", bufs=2) as xtpool,
            tc.tile_pool(name="p1dbl", bufs=2) as dpool,
            tc.tile_pool(name="p1sgl", bufs=1) as spool,
            tc.tile_pool(name="p1ps", bufs=4, space="PSUM") as ppool,
        ):
            bhl_sb = wpool.tile([2, D_OUT], f16)
            nc.sync.dma_start(bhl_sb[0:1, :], bench.ap())
            nc.sync.dma_start(bhl_sb[1:2, :], bencl.ap())
            ones_sb = wpool.tile([2, P], f16)
            nc.vector.memset(ones_sb[:], 1.0)
            wh_s = []
            wl_s = []
            for s in range(NSL):
                wh = wpool.tile([P, KO, SL], f16, name=f"whs{s}")
                nc.sync.dma_start(
                    wh[:], wencTh.ap().rearrange("(o p) f -> p o f", p=P)[:, :, ts(s, SL)]
                )
                wh_s.append(wh)
                wl = wpool.tile([P, KO, SL], f16, name=f"wls{s}")
                nc.sync.dma_start(
                    wl[:], wencTl.ap().rearrange("(o p) f -> p o f", p=P)[:, :, ts(s, SL)]
                )
                wl_s.append(wl)
            f16_dmas = []
            for b in range(nblk):
                xhl = xtpool.tile([P, 2 * KO, P], f16, name=f"xhl{b}", tag="xhl")
                nc.scalar.dma_start(xhl[:], xThl_t[:, :, ts(b, P)])
                xh = xhl[:, :KO, :]
                xl = xhl[:, KO:, :]

                pre = dpool.tile([P, D_OUT], f32, name=f"pre{b}", tag="pre")
                sq = dpool.tile([P, D_OUT], f32, name=f"sq{b}", tag="sq")
                for s in range(NSL):
                    ps = ppool.tile([P, SL], f32, name=f"eps{b}_{s}", tag="eps")
                    nc.tensor.matmul(
                        ps[:], ones_sb[:], bhl_sb[:, ts(s, SL)], start=True, stop=False
                    )
                    for pi, (xt_c, w_c) in enumerate(
                        ((xh, wh_s[s]), (xh, wl_s[s]), (xl, wh_s[s]))
                    ):
                        for ko in range(KO):
                            nc.tensor.matmul(
                                ps[:],
                                xt_c[:, ko, :],
                                w_c[:, ko, :],
                                start=False,
                                stop=(pi == 2 and ko == KO - 1),
                            )
                    nc.scalar.activation(
                        sq[:, ts(s, SL)], ps[:], mybir.ActivationFunctionType.Square
                    )
                    nc.scalar.copy(pre[:, ts(s, SL)], ps[:])

                # top-32 threshold (on squares): 4x max8 + 3x match_replace.
                # Round-1 max8 runs split in halves so the first half starts
                # as soon as encoder slices 0-3 are evacuated.
                zap = spool.tile([P, D_OUT], f32, name=f"zap{b}", tag="zap")
                m8 = dpool.tile([P, 4, 8], f32, name=f"m8{b}", tag="m8")
                m16 = dpool.tile([P, 16], f32, name=f"m16{b}", tag="m16")
                nc.vector.max(out=m16[:, :8], in_=sq[:, : D_OUT // 2])
                nc.vector.max(out=m16[:, 8:], in_=sq[:, D_OUT // 2 :])
                nc.vector.max(out=m8[:, 0, :], in_=m16[:])
                nc.vector.match_replace(
                    out=zap[:], in_to_replace=m8[:, 0, :], in_values=sq[:], imm_value=-1.0
                )
                for r in range(1, 4):
                    nc.vector.max(out=m8[:, r, :], in_=zap[:])
                    if r < 3:
                        nc.vector.match_replace(
                            out=zap[:],
                            in_to_replace=m8[:, r, :],
                            in_values=zap[:],
                            imm_value=-1.0,
                        )

                f_sb = pre
                nc.vector.scalar_tensor_tensor(
                    out=f_sb[:],
                    in0=sq[:],
                    scalar=m8[:, 3, 7:8],
                    in1=pre[:],
                    op0=mybir.AluOpType.is_ge,
                    op1=mybir.AluOpType.mult,
                )
                nc.scalar.dma_start(f_out.ap()[ts(b, P), :], f_sb[:])
                f16t = spool.tile([P, D_OUT], f16, name=f"f16t{b}", tag="f16t")
                nc.scalar.copy(f16t[:], f_sb[:])
                f16_dma = nc.scalar.dma_start(f16_scr.ap()[ts(b, P), :], f16t[:])
                f16_dmas.append(f16_dma)

        # ---------------- Phase 2: decode ----------------
        with (
            tc.tile_pool(name="p2w", bufs=1) as wpool2,
            tc.tile_pool(name="p2ft", bufs=3) as ftpool,
            tc.tile_pool(name="p2rec", bufs=3) as recpool,
            tc.tile_pool(name="p2ps", bufs=4, space="PSUM") as rpspool,
        ):
            wdec_g = []
            for g4 in range(4):
                wg = wpool2.tile([P, FC // 4, D_IN], f16, name=f"wdg{g4}")
                nc.sync.dma_start(
                    wg[:],
                    wdecT.ap().rearrange("(o p) d -> p o d", p=P)[
                        :, ts(g4, FC // 4), :
                    ],
                )
                wdec_g.append(wg)
            bdhl_sb = wpool2.tile([2, D_IN], f16)
            nc.sync.dma_start(bdhl_sb[0:1, :], bdech.ap())
            nc.sync.dma_start(bdhl_sb[1:2, :], bdecl.ap())
            ones2 = wpool2.tile([2, P], f16)
            nc.vector.memset(ones2[:], 1.0)

            for b in range(nblk):
                fT = ftpool.tile([P, FC, P], f16, name=f"fT{b}", tag="fT")
                tr_eng = nc.sync if b % 2 == 0 else nc.scalar
                tr = tr_eng.dma_start_transpose(fT[:], f16_scr.ap()[ts(b, P), :])
                add_dep_helper(
                    tr.ins, f16_dmas[b].ins, sync=True, reason="f16 scratch RAW"
                )

                rps = rpspool.tile([P, D_IN], f32, name=f"rps{b}", tag="rps")
                # chunk-outer: both output ranges stream from one stationary
                nc.tensor.matmul(
                    rps[:, 0:512], ones2[:], bdhl_sb[:, 0:512], start=True, stop=False
                )
                nc.tensor.matmul(
                    rps[:, 512:768], ones2[:], bdhl_sb[:, 512:768], start=True, stop=False
                )
                for c in range(FC):
                    for n0, n1 in ((0, 512), (512, 768)):
                        nc.tensor.matmul(
                            rps[:, n0:n1],
                            fT[:, c, :],
                            wdec_g[c // 8][:, c % 8, n0:n1],
                            start=False,
                            stop=(c == FC - 1),
                        )
                rec = recpool.tile([P, D_IN], f32, name=f"rec{b}", tag="rec")
                nc.scalar.copy(rec[:], rps[:])
                nc.scalar.dma_start(recon_out.ap()[ts(b, P), :], rec[:])

    nc.compile()
    return nc


_BUILT = {}


def _get_built(t_core: int):
    if t_core not in _BUILT:
        _BUILT[t_core] = build(t_core)
    return _BUILT[t_core]


def _split16(a):
    hi = a.astype(np.float16)
    lo = (a - hi.astype(np.float32)).astype(np.float16)
    return hi, lo


def _install_ntff_shim():
    """The image's antenv lacks axon_hooks; synthesize it from trn_agent_boot
    so run_bass_kernel_spmd(trace=True) can capture NTFF profiles."""
    import types

    if "antenv.axon_hooks" in sys.modules:
        return
    try:
        from trn_agent_boot.trn_boot import _ntff_profile_via_ctypes

        hook = _ntff_profile_via_ctypes("/opt/axon/libaxon_pjrt.so")
        mod = types.ModuleType("antenv.axon_hooks")
        mod.get_axon_ntff_profile_hook = lambda: hook
        sys.modules["antenv.axon_hooks"] = mod
    except Exception:
        pass


def _enable_ldw_opt():
    from concourse import bass_utils as bu

    if getattr(bu, "_ldw_opt_patched", False):
        return
    orig = bu.run_command

    def patched(cmd, *a, **kw):
        if isinstance(cmd, list):
            cmd = [
                "--enable-ldw-opt=true" if c == "--enable-ldw-opt=false" else c
                for c in cmd
            ]
        return orig(cmd, *a, **kw)

    bu.run_command = patched
    bu._ldw_opt_patched = True


def kernel(x, W_enc, b_enc, W_dec, b_dec):
    global LAST_RESULTS
    from concourse.bass_utils import run_bass_kernel_spmd

    if os.environ.get("SAE_LDW_OPT"):
        _enable_ldw_opt()

    if os.environ.get("SAE_TRACE"):
        _install_ntff_shim()

    x = np.asarray(x, dtype=np.float32)
    W_enc = np.asarray(W_enc, dtype=np.float32)
    b_enc = np.asarray(b_enc, dtype=np.float32)
    W_dec = np.asarray(W_dec, dtype=np.float32)
    b_dec = np.asarray(b_dec, dtype=np.float32)

    n_tokens = x.shape[0]
    t_core = n_tokens // N_CORES
    nc = _get_built(t_core)

    xT = np.ascontiguousarray(x.T)  # [768, N]
    xTh, xTl = _split16(xT)
    xThl = np.concatenate([xTh, xTl], axis=0)  # [1536, N]
    wencTh, wencTl = _split16(np.ascontiguousarray(W_enc.T))
    bench_, bencl_ = _split16(b_enc[None, :])
    wdecT = np.ascontiguousarray(W_dec.T).astype(np.float16)
    bdech_, bdecl_ = _split16(b_dec[None, :])

    in_maps = [
        {
            "xThl": np.ascontiguousarray(xThl[:, i * t_core : (i + 1) * t_core]),
            "wencTh": wencTh,
            "wencTl": wencTl,
            "bench": bench_,
            "bencl": bencl_,
            "wdecT": wdecT,
            "bdech": bdech_,
            "bdecl": bdecl_,
        }
        for i in range(N_CORES)
    ]

    res = run_bass_kernel_spmd(
        nc,
        in_maps,
        list(range(N_CORES)),
        trace=bool(os.environ.get("SAE_TRACE")),
    )
    LAST_RESULTS = res

    recon = np.concatenate([res.results[i]["recon"] for i in range(N_CORES)], axis=0)
    f = np.concatenate([res.results[i]["f"] for i in range(N_CORES)], axis=0)
    return recon, f


# revision 17
# speedup vs baseline: 1.0428x; 1.0026x over previous
"""Trainium2 Bass kernel for BipolarSAE (top-k masking sparse autoencoder).

reference:
    pre = x @ W_enc.T + b_enc          # [N, 4096]
    keep top-32 of |pre| per row (mask), f = pre * mask
    recon = f @ W_dec.T + b_dec        # [N, 768]
    returns (recon, f)

Strategy (8 NeuronCores, data-parallel over the 32768 tokens, 4096 each):
  Phase 1 (W_enc resident): encoder matmul via fp16 hi/lo decomposition —
    x = x_hi + x_lo, W = W_hi + W_lo (fp16 splits, products exact in fp32
    PSUM accumulation), pre = x_hi@W_hi + x_hi@W_lo + x_lo@W_hi. Error
    ~2^-22 relative, matching native fp32, at 3 bf16-rate passes instead
    of fp32's 4 cycles/row. Selection needs this precision: the top-32
    boundary gap can be ~1e-7.
    ACT evacuates PSUM as pre (fp32) and pre^2; VectorE extracts the
    32nd-largest square via 4x max8 + 3x match_replace, then one
    scalar_tensor_tensor applies the threshold mask: f = (sq>=tau^2)*pre.
    f goes to DRAM in fp32 (output) and fp16 (scratch for phase 2).
  Phase 2 (W_dec resident, fp16): per block one DMA-transpose loads
    f^T (features-on-partitions) from the fp16 scratch; decoder matmul
    all-fp16 (decoder precision is not selection-critical).
Biases are folded in as K=1 matmuls with a ones row-vector (b_enc in
fp16 hi+lo for exactness; b_dec likewise).
"""

import os
import sys

sys.path.insert(0, "/opt/trn_rl_repo")

import numpy as np

import concourse.bacc as bacc
import concourse.bass as bass
import concourse.mybir as mybir
import concourse.tile as tile
from concourse.bass import ts
from concourse.tile_rust import add_dep_helper

P = 128
D_IN = 768
D_OUT = 4096
K_TOP = 32
N_TOKENS = 32768
N_CORES = 8

KO = D_IN // P  # 6 contraction chunks (encoder)
NSL = 8  # encoder feature slices
SL = D_OUT // NSL  # 512
FC = D_OUT // P  # 32 feature chunks (decoder contraction)

LAST_RESULTS = None  # test harness reads exec_time_ns from here

f32 = mybir.dt.float32
f16 = mybir.dt.float16


def build(t_core: int) -> bacc.Bacc:
    nblk = t_core // P
    nc = bacc.Bacc("TRN2", target_bir_lowering=False, debug=False)

    xThl = nc.declare_dram_parameter("xThl", [2 * D_IN, t_core], f16, isOutput=False)
    wencTh = nc.declare_dram_parameter("wencTh", [D_IN, D_OUT], f16, isOutput=False)
    wencTl = nc.declare_dram_parameter("wencTl", [D_IN, D_OUT], f16, isOutput=False)
    bench = nc.declare_dram_parameter("bench", [1, D_OUT], f16, isOutput=False)
    bencl = nc.declare_dram_parameter("bencl", [1, D_OUT], f16, isOutput=False)
    wdecT = nc.declare_dram_parameter("wdecT", [D_OUT, D_IN], f16, isOutput=False)
    bdech = nc.declare_dram_parameter("bdech", [1, D_IN], f16, isOutput=False)
    bdecl = nc.declare_dram_parameter("bdecl", [1, D_IN], f16, isOutput=False)

    f_out = nc.declare_dram_parameter("f", [t_core, D_OUT], f32, isOutput=True)
    f16_scr = nc.dram_tensor("f16scr", [t_core, D_OUT], f16)
    recon_out = nc.declare_dram_parameter("recon", [t_core, D_IN], f32, isOutput=True)

    xThl_t = xThl.ap().rearrange("(o p) t -> p o t", p=P)  # [128, 12, t]

    with tile.TileContext(nc) as tc:
        # ---------------- Phase 1: encode + top-k mask ----------------
        with (
            tc.tile_pool(name="p1w", bufs=1) as wpool,
            tc.tile_pool(name="p1xt", bufs=2) as xtpool,
            tc.tile_pool(name="p1dbl", bufs=2) as dpool,
            tc.tile_pool(name="p1sgl", bufs=1) as spool,
            tc.tile_pool(name="p1ps", bufs=4, space="PSUM") as ppool,
        ):
            bhl_sb = wpool.tile([2, D_OUT], f16)
            nc.sync.dma_start(bhl_sb[0:1, :], bench.ap())
            nc.sync.dma_start(bhl_sb[1:2, :], bencl.ap())
            ones_sb = wpool.tile([2, P], f16)
            nc.vector.memset(ones_sb[:], 1.0)
            wh_s = []
            wl_s = []
            for s in range(NSL):
                wh = wpool.tile([P, KO, SL], f16, name=f"whs{s}")
                nc.sync.dma_start(
                    wh[:], wencTh.ap().rearrange("(o p) f -> p o f", p=P)[:, :, ts(s, SL)]
                )
                wh_s.append(wh)
                wl = wpool.tile([P, KO, SL], f16, name=f"wls{s}")
                nc.sync.dma_start(
                    wl[:], wencTl.ap().rearrange("(o p) f -> p o f", p=P)[:, :, ts(s, SL)]
                )
                wl_s.append(wl)
            f16_dmas = []
            for b in range(nblk):
                xhl = xtpool.tile([P, 2 * KO, P], f16, name=f"xhl{b}", tag="xhl")
                nc.scalar.dma_start(xhl[:], xThl_t[:, :, ts(b, P)])
                xh = xhl[:, :KO, :]
                xl = xhl[:, KO:, :]

                pre = dpool.tile([P, D_OUT], f32, name=f"pre{b}", tag="pre")
                sq = dpool.tile([P, D_OUT], f32, name=f"sq{b}", tag="sq")
                for s in range(NSL):
                    ps = ppool.tile([P, SL], f32, name=f"eps{b}_{s}", tag="eps")
                    nc.tensor.matmul(
                        ps[:], ones_sb[:], bhl_sb[:, ts(s, SL)], start=True, stop=False
                    )
                    for pi, (xt_c, w_c) in enumerate(
                        ((xh, wh_s[s]), (xh, wl_s[s]), (xl, wh_s[s]))
                    ):
                        for ko in range(KO):
                            nc.tensor.matmul(
                                ps[:],
                                xt_c[:, ko, :],
                                w_c[:, ko, :],
                                start=False,
                                stop=(pi == 2 and ko == KO - 1),
                            )
                    nc.scalar.activation(
                        sq[:, ts(s, SL)], ps[:], mybir.ActivationFunctionType.Square
                    )
                    nc.scalar.copy(pre[:, ts(s, SL)], ps[:])

                # top-32 threshold (on squares): 4x max8 + 3x match_replace.
                # Round-1 max8 runs split in halves so the first half starts
                # as soon as encoder slices 0-3 are evacuated.
                zap = spool.tile([P, D_OUT], f32, name=f"zap{b}", tag="zap")
                m8 = dpool.tile([P, 4, 8], f32, name=f"m8{b}", tag="m8")
                m16 = dpool.tile([P, 16], f32, name=f"m16{b}", tag="m16")
                nc.vector.max(out=m16[:, :8], in_=sq[:, : D_OUT // 2])
                nc.vector.max(out=m16[:, 8:], in_=sq[:, D_OUT // 2 :])
                nc.vector.max(out=m8[:, 0, :], in_=m16[:])
                nc.vector.match_replace(
                    out=zap[:], in_to_replace=m8[:, 0, :], in_values=sq[:], imm_value=-1.0
                )
                for r in range(1, 4):
                    nc.vector.max(out=m8[:, r, :], in_=zap[:])
                    if r < 3:
                        nc.vector.match_replace(
                            out=zap[:],
                            in_to_replace=m8[:, r, :],
                            in_values=zap[:],
                            imm_value=-1.0,
                        )

                f_sb = pre
                nc.vector.scalar_tensor_tensor(
                    out=f_sb[:],
                    in0=sq[:],
                    scalar=m8[:, 3, 7:8],
                    in1=pre[:],
                    op0=mybir.AluOpType.is_ge,
                    op1=mybir.AluOpType.mult,
                )
                nc.scalar.dma_start(f_out.ap()[ts(b, P), :], f_sb[:])
                f16t = spool.tile([P, D_OUT], f16, name=f"f16t{b}", tag="f16t")
                nc.scalar.copy(f16t[:], f_sb[:])
                f16_dma = nc.scalar.dma_start(f16_scr.ap()[ts(b, P), :], f16t[:])
                f16_dmas.append(f16_dma)

        # ---------------- Phase 2: decode ----------------
        with (
            tc.tile_pool(name="p2w", bufs=1) as wpool2,
            tc.tile_pool(name="p2ft", bufs=3) as ftpool,
            tc.tile_pool(name="p2rec", bufs=3) as recpool,
            tc.tile_pool(name="p2ps", bufs=4, space="PSUM") as rpspool,
        ):
            wdec_g = []
            for g4 in range(4):
                wg = wpool2.tile([P, FC // 4, D_IN], f16, name=f"wdg{g4}")
                nc.sync.dma_start(
                    wg[:],
                    wdecT.ap().rearrange("(o p) d -> p o d", p=P)[
                        :, ts(g4, FC // 4), :
                    ],
                )
                wdec_g.append(wg)
            bdhl_sb = wpool2.tile([2, D_IN], f16)
            nc.sync.dma_start(bdhl_sb[0:1, :], bdech.ap())
            nc.sync.dma_start(bdhl_sb[1:2, :], bdecl.ap())
            ones2 = wpool2.tile([2, P], f16)
            nc.vector.memset(ones2[:], 1.0)

            for b in range(nblk):
                fT = ftpool.tile([P, FC, P], f16, name=f"fT{b}", tag="fT")
                tr_eng = nc.sync if b % 2 == 0 else nc.scalar
                tr = tr_eng.dma_start_transpose(fT[:], f16_scr.ap()[ts(b, P), :])
                add_dep_helper(
                    tr.ins, f16_dmas[b].ins, sync=True, reason="f16 scratch RAW"
                )

                rps = rpspool.tile([P, D_IN], f32, name=f"rps{b}", tag="rps")
                # chunk-outer: both output ranges stream from one stationary
                nc.tensor.matmul(
                    rps[:, 0:512], ones2[:], bdhl_sb[:, 0:512], start=True, stop=False
                )
                nc.tensor.matmul(
                    rps[:, 512:768], ones2[:], bdhl_sb[:, 512:768], start=True, stop=False
                )
                for c in range(FC):
                    for n0, n1 in ((0, 512), (512, 768)):
                        nc.tensor.matmul(
                            rps[:, n0:n1],
                            fT[:, c, :],
                            wdec_g[c // 8][:, c % 8, n0:n1],
                            start=False,
                            stop=(c == FC - 1),
                        )
                rec = recpool.tile([P, D_IN], f32, name=f"rec{b}", tag="rec")
                nc.scalar.copy(rec[:], rps[:])
                nc.scalar.dma_start(recon_out.ap()[ts(b, P), :], rec[:])

    nc.compile()
    return nc


_BUILT = {}


def _get_built(t_core: int):
    if t_core not in _BUILT:
        _BUILT[t_core] = build(t_core)
    return _BUILT[t_core]


def _split16(a):
    hi = a.astype(np.float16)
    lo = (a - hi.astype(np.float32)).astype(np.float16)
    return hi, lo


def _install_ntff_shim():
    """The image's antenv lacks axon_hooks; synthesize it from trn_agent_boot
    so run_bass_kernel_spmd(trace=True) can capture NTFF profiles."""
    import types

    if "antenv.axon_hooks" in sys.modules:
        return
    try:
        from trn_agent_boot.trn_boot import _ntff_profile_via_ctypes

        hook = _ntff_profile_via_ctypes("/opt/axon/libaxon_pjrt.so")
        mod = types.ModuleType("antenv.axon_hooks")
        mod.get_axon_ntff_profile_hook = lambda: hook
        sys.modules["antenv.axon_hooks"] = mod
    except Exception:
        pass


def _enable_ldw_opt():
    from concourse import bass_utils as bu

    if getattr(bu, "_ldw_opt_patched", False):
        return
    orig = bu.run_command

    def patched(cmd, *a, **kw):
        if isinstance(cmd, list):
            cmd = [
                "--enable-ldw-opt=true" if c == "--enable-ldw-opt=false" else c
                for c in cmd
            ]
        return orig(cmd, *a, **kw)

    bu.run_command = patched
    bu._ldw_opt_patched = True


def kernel(x, W_enc, b_enc, W_dec, b_dec):
    global LAST_RESULTS
    from concourse.bass_utils import run_bass_kernel_spmd

    if os.environ.get("SAE_LDW_OPT"):
        _enable_ldw_opt()

    if os.environ.get("SAE_TRACE"):
        _install_ntff_shim()

    x = np.asarray(x, dtype=np.float32)
    W_enc = np.asarray(W_enc, dtype=np.float32)
    b_enc = np.asarray(b_enc, dtype=np.float32)
    W_dec = np.asarray(W_dec, dtype=np.float32)
    b_dec = np.asarray(b_dec, dtype=np.float32)

    n_tokens = x.shape[0]
    t_core = n_tokens // N_CORES
    nc = _get_built(t_core)

    xT = np.ascontiguousarray(x.T)  # [768, N]
    xTh, xTl = _split16(xT)
    xThl = np.concatenate([xTh, xTl], axis=0)  # [1536, N]
    wencTh, wencTl = _split16(np.ascontiguousarray(W_enc.T))
    bench_, bencl_ = _split16(b_enc[None, :])
    wdecT = np.ascontiguousarray(W_dec.T).astype(np.float16)
    bdech_, bdecl_ = _split16(b_dec[None, :])

    in_maps = [
        {
            "xThl": np.ascontiguousarray(xThl[:, i * t_core : (i + 1) * t_core]),
            "wencTh": wencTh,
            "wencTl": wencTl,
            "bench": bench_,
            "bencl": bencl_,
            "wdecT": wdecT,
            "bdech": bdech_,
            "bdecl": bdecl_,
        }
        for i in range(N_CORES)
    ]

    res = run_bass_kernel_spmd(
        nc,
        in_maps,
        list(range(N_CORES)),
        trace=bool(os.environ.get("SAE_TRACE")),
    )
    LAST_RESULTS = res

    recon = np.concatenate([res.results[i]["recon"] for i in range(N_CORES)], axis=0)
    f = np.concatenate([res.results[i]["f"] for i in range(N_CORES)], axis=0)
    return recon, f
